# revision 4
# baseline (speedup 1.0000x reference)
"""Two-layer GCN (contrastive head) on 8 Trainium2 NeuronCores.

Strategy (graph/data parallel per the node-partition sharding):
  * Host: compute degree normalizations, build a load-balancing node
    permutation (slots), partition edges by owner of dst, group per
    (dst-tile, src-half) into fixed-capacity chunk segments.
  * Device (SPMD on 8 cores):
      L1 transform:  h = (x * rsqrt(deg_out)) @ W1      (own nodes, PE fp32)
      AllGather bf16 h-table  -> every core holds the full table
      L1 aggregate:  dma_gather(h[src]) + one-hot segment-sum on PE
      h1 = relu(agg * rsqrt(deg_in) + b1); fold * rsqrt(deg_out)
      L2 transform:  t2 = h1s @ W2 ; AllGather bf16 t2-table
      L2 aggregate:  gather + segment-sum;  hidden = agg2 * rsqrt(deg_in) + b2
      logits = hidden @ Wf + bf
  * Host: concat per-core shards, un-permute, trim padding.
"""

import numpy as np
import ml_dtypes

import concourse.bacc as bacc
import concourse.tile as tile
from concourse import mybir
from concourse.bass_utils import run_bass_kernel_spmd

N_CORES = 8
TILE = 128
PAD_SLOT_VAL = 200.0  # one-hot compare value that never matches iota 0..127
GATHER_MAX_CHUNKS = 4  # 8*128=1024 idxs -> 66 descs/engine fits the SWDGE ring
                       # (1024 verified OK on HW, 1536 crashes the Q7 decode)


# --------------------------------------------------------------------------
# host-side preprocessing
# --------------------------------------------------------------------------

def _balance_slots(deg_in, n_bins):
    """Assign each node to a (core,tile) bin of capacity TILE, balancing
    total in-degree per bin (greedy LPT).  Returns slot_of[node]."""
    import heapq

    n = deg_in.shape[0]
    order = np.argsort(-deg_in, kind="stable")
    heap = [(0, b) for b in range(n_bins)]
    heapq.heapify(heap)
    counts = np.zeros(n_bins, np.int64)
    slot_of = np.empty(n, np.int64)
    for v in order:
        load, b = heapq.heappop(heap)
        slot_of[v] = b * TILE + counts[b]
        counts[b] += 1
        if counts[b] < TILE:
            heapq.heappush(heap, (load + int(deg_in[v]), b))
    return slot_of


def _wrap_idx(flat):
    """[n_seg, cap] int -> SWDGE wrapped layout [n_seg, 128, cap//16] int16
    (index i at partition i%16, column i//16; replicated x8 down partitions)."""
    n_seg, cap = flat.shape
    w = flat.reshape(n_seg, cap // 16, 16).transpose(0, 2, 1).astype(np.int16)
    return np.tile(w, (1, 8, 1))


def host_prep(x, W1, b1, W2, b2, Wf, bf, src, dst):
    n, in_f = x.shape
    hid_f = W1.shape[1]
    out_f = W2.shape[1]
    spc = -(-n // (N_CORES * TILE)) * TILE          # slots per core
    S = N_CORES * spc
    half = S // 2
    tpc = spc // TILE                                # tiles per core

    deg_out = np.bincount(src, minlength=n)
    deg_in = np.bincount(dst, minlength=n)
    s_out = 1.0 / np.sqrt(np.maximum(deg_out, 1)).astype(np.float32)
    s_in = 1.0 / np.sqrt(np.maximum(deg_in, 1)).astype(np.float32)

    slot_of = _balance_slots(deg_in, S // TILE)

    ds = slot_of[dst]
    ss = slot_of[src]
    half_e = (ss >= half).astype(np.int64)
    gidx = ss - half_e * half
    seg = (ds // TILE) * 2 + half_e                  # global segment id
    n_seg = (S // TILE) * 2

    cnt = np.bincount(seg, minlength=n_seg)
    K = max(1, -(-int(cnt.max()) // TILE))           # chunks per segment
    cap = K * TILE

    starts = np.zeros(n_seg + 1, np.int64)
    np.cumsum(cnt, out=starts[1:])
    eorder = np.argsort(seg, kind="stable")
    pos = np.arange(len(src)) - starts[seg[eorder]]

    idx_pad = np.zeros((n_seg, cap), np.int64)
    slot_pad = np.full((n_seg, cap), PAD_SLOT_VAL, np.float32)
    idx_pad[seg[eorder], pos] = gidx[eorder]
    slot_pad[seg[eorder], pos] = (ds % TILE)[eorder]

    # per-core device inputs
    xs = (x * s_out[:, None]).astype(np.float32)
    xp = np.zeros((S, in_f), np.float32)
    xp[slot_of] = xs
    s_in_sl = np.ones(S, np.float32)
    s_in_sl[slot_of] = s_in
    s_out_sl = np.ones(S, np.float32)
    s_out_sl[slot_of] = s_out
    sc1_sl = s_in_sl * s_out_sl

    iota = np.tile(np.arange(TILE, dtype=np.float32), (TILE, 1)).astype(
        ml_dtypes.bfloat16
    )
    ident = np.eye(TILE, dtype=np.float32)

    per_core = []
    for c in range(N_CORES):
        seg_lo, seg_hi = c * tpc * 2, (c + 1) * tpc * 2
        m = {
            "xT": np.ascontiguousarray(xp[c * spc:(c + 1) * spc].T),
            "w1": np.asarray(W1, np.float32),
            "w2": np.asarray(W2, np.float32),
            "wf": np.asarray(Wf, np.float32),
            "gidx": np.ascontiguousarray(_wrap_idx(idx_pad[seg_lo:seg_hi])),
            "slots": np.ascontiguousarray(
                slot_pad[seg_lo:seg_hi]
                .reshape(tpc, 2, K, TILE)
                .transpose(0, 3, 1, 2)
                .reshape(tpc, TILE, 2 * K)
                .astype(np.float32)
            ),
            "sin": np.ascontiguousarray(
                s_in_sl[c * spc:(c + 1) * spc].reshape(tpc, TILE).T
            ),
            "sc1": np.ascontiguousarray(
                sc1_sl[c * spc:(c + 1) * spc].reshape(tpc, TILE).T
            ),
            "iota": iota,
            "ident": ident,
        }
        if np.any(b1):
            m["b1t"] = np.tile(np.asarray(b1, np.float32), (TILE, 1))
            m["soutc"] = np.ascontiguousarray(
                s_out_sl[c * spc:(c + 1) * spc].reshape(tpc, TILE).T
            )
        if np.any(b2):
            m["b2t"] = np.tile(np.asarray(b2, np.float32), (TILE, 1))
        if np.any(bf):
            m["bft"] = np.tile(np.asarray(bf, np.float32), (TILE, 1))
        per_core.append(m)

    meta = dict(
        n=n, in_f=in_f, hid_f=hid_f, out_f=out_f, spc=spc, S=S, half=half,
        tpc=tpc, K=K, slot_of=slot_of,
        has_b1=bool(np.any(b1)), has_b2=bool(np.any(b2)), has_bf=bool(np.any(bf)),
    )
    return per_core, meta


# --------------------------------------------------------------------------
# device program
# --------------------------------------------------------------------------

def build_program(meta, reps=1):
    in_f, hid_f, out_f = meta["in_f"], meta["hid_f"], meta["out_f"]
    spc, S, half, tpc, K = (
        meta["spc"], meta["S"], meta["half"], meta["tpc"], meta["K"],
    )
    has_b1, has_b2, has_bf = meta["has_b1"], meta["has_b2"], meta["has_bf"]
    lgt_f = 2
    kin = in_f // TILE                      # k-tiles of the L1 transform
    assert in_f % TILE == 0 and hid_f == TILE and out_f <= TILE

    f32, bf16, i16 = mybir.dt.float32, mybir.dt.bfloat16, mybir.dt.int16
    nc = bacc.Bacc("TRN2", target_bir_lowering=False, debug=False,
                   num_devices=N_CORES)

    def din(name, shape, dt):
        return nc.dram_tensor(name, shape, dt, kind="ExternalInput").ap()

    xT = din("xT", [in_f, spc], f32)
    w1 = din("w1", [in_f, hid_f], f32)
    w2 = din("w2", [hid_f, out_f], f32)
    wf = din("wf", [out_f, lgt_f], f32)
    gidx = din("gidx", [2 * tpc, TILE, K * 8], i16)
    slots = din("slots", [tpc, TILE, 2 * K], f32)
    sin_d = din("sin", [TILE, tpc], f32)
    sc1_d = din("sc1", [TILE, tpc], f32)
    iota_d = din("iota", [TILE, TILE], bf16)
    ident_d = din("ident", [TILE, TILE], f32)
    b1t_d = din("b1t", [TILE, hid_f], f32) if has_b1 else None
    soutc_d = din("soutc", [TILE, tpc], f32) if has_b1 else None
    b2t_d = din("b2t", [TILE, out_f], f32) if has_b2 else None
    bft_d = din("bft", [TILE, lgt_f], f32) if has_bf else None

    hid_out = nc.dram_tensor("hid_out", [spc, out_f], f32,
                             kind="ExternalOutput").ap()
    log_out = nc.dram_tensor("log_out", [spc, lgt_f], f32,
                             kind="ExternalOutput").ap()

    shard1 = nc.dram_tensor("shard1", [spc, hid_f], bf16).ap()
    table1 = nc.dram_tensor("table1", [S, hid_f], bf16, addr_space="Shared").ap()
    shard2 = nc.dram_tensor("shard2", [spc, TILE], bf16).ap()
    table2 = nc.dram_tensor("table2", [S, TILE], bf16, addr_space="Shared").ap()
    groups = [list(range(N_CORES))]

    # gather spans per segment (split if K exceeds the SWDGE ring capacity)
    spans = []
    j0 = 0
    while j0 < K:
        j1 = min(j0 + GATHER_MAX_CHUNKS, K)
        spans.append((j0, j1))
        j0 = j1

    with tile.TileContext(nc) as tc:
        with (
            tc.tile_pool(name="const", bufs=1) as cpool,
            tc.tile_pool(name="io", bufs=3) as io,
            tc.tile_pool(name="gth", bufs=3) as gpool,
            tc.tile_pool(name="onehot", bufs=4) as spool,
            tc.tile_pool(name="work", bufs=3) as wpool,
            tc.tile_pool(name="psA", bufs=2, space="PSUM") as psA,
            tc.tile_pool(name="psB", bufs=2, space="PSUM") as psB,
        ):
            iota_t = cpool.tile([TILE, TILE], bf16, tag="iota")
            nc.sync.dma_start(iota_t[:], iota_d[:])
            ident_t = cpool.tile([TILE, TILE], f32, tag="ident")
            nc.sync.dma_start(ident_t[:], ident_d[:])
            w1_t = []
            for k in range(kin):
                t = cpool.tile([TILE, hid_f], f32, tag=f"w1_{k}")
                nc.sync.dma_start(t[:], w1[k * TILE:(k + 1) * TILE, :])
                w1_t.append(t)
            w2_t = cpool.tile([hid_f, out_f], f32, tag="w2")
            nc.sync.dma_start(w2_t[:], w2[:])
            wf_t = cpool.tile([out_f, lgt_f], f32, tag="wf")
            nc.sync.dma_start(wf_t[:], wf[:])
            sin_t = cpool.tile([TILE, tpc], f32, tag="sin")
            nc.sync.dma_start(sin_t[:], sin_d[:])
            sc1_t = cpool.tile([TILE, tpc], f32, tag="sc1")
            nc.sync.dma_start(sc1_t[:], sc1_d[:])
            if has_b1:
                b1_t = cpool.tile([TILE, hid_f], f32, tag="b1")
                nc.sync.dma_start(b1_t[:], b1t_d[:])
                soutc_t = cpool.tile([TILE, tpc], f32, tag="soutc")
                nc.sync.dma_start(soutc_t[:], soutc_d[:])
            if has_b2:
                b2_t = cpool.tile([TILE, out_f], f32, tag="b2")
                nc.sync.dma_start(b2_t[:], b2t_d[:])
            if has_bf:
                bf_t = cpool.tile([TILE, lgt_f], f32, tag="bf")
                nc.sync.dma_start(bf_t[:], bft_d[:])

            def gather_tile(t, table):
                """Issue the 2 half-gathers for dst-tile t. Returns tiles."""
                gts = []
                for h in (0, 1):
                    it = io.tile([TILE, K * 8], i16, tag="it")
                    nc.sync.dma_start(it[:], gidx[2 * t + h])
                    gt = gpool.tile([TILE, K, TILE], bf16, tag="gt")
                    for (j0, j1) in spans:
                        nidx = (j1 - j0) * TILE
                        nc.gpsimd.dma_gather(
                            gt[:, j0:j1, :],
                            table[h * half:(h + 1) * half, :],
                            it[:, j0 * 8:j1 * 8],
                            nidx, nidx, TILE,
                        )
                    gts.append(gt)
                return gts

            def segsum(t, gts, st, fdim):
                """PSUM accumulate one-hot.T @ gathered over 2K chunks."""
                ps = psA.tile([TILE, fdim], f32, tag="agg")
                for j in range(2 * K):
                    s_t = spool.tile([TILE, TILE], bf16, tag="oh")
                    nc.vector.tensor_scalar(
                        out=s_t[:], in0=iota_t[:], scalar1=st[:, j:j + 1],
                        scalar2=None, op0=mybir.AluOpType.is_equal,
                    )
                    nc.tensor.matmul(
                        ps[:], lhsT=s_t[:], rhs=gts[j // K][:, j % K, 0:fdim],
                        start=(j == 0), stop=(j == 2 * K - 1),
                    )
                return ps

            for _ in range(reps):
                # ---- L1 transform: own nodes ----
                for t in range(tpc):
                    ps = psB.tile([TILE, hid_f], f32, tag="mm")
                    for k in range(kin):
                        xt = io.tile([TILE, TILE], f32, tag="xt")
                        nc.sync.dma_start(
                            xt[:],
                            xT[k * TILE:(k + 1) * TILE, t * TILE:(t + 1) * TILE],
                        )
                        nc.tensor.matmul(ps[:], lhsT=xt[:], rhs=w1_t[k][:],
                                         start=(k == 0), stop=(k == kin - 1))
                    sh = wpool.tile([TILE, hid_f], bf16, tag="sh1")
                    nc.vector.tensor_copy(sh[:], ps[:])
                    nc.sync.dma_start(shard1[t * TILE:(t + 1) * TILE, :], sh[:])

                nc.gpsimd.collective_compute(
                    "AllGather", mybir.AluOpType.bypass, replica_groups=groups,
                    ins=[shard1[:]], outs=[table1[:]],
                )

                # ---- L1 aggregate + h1 + L2 transform ----
                for t in range(tpc):
                    gts = gather_tile(t, table1)
                    st = io.tile([TILE, 2 * K], f32, tag="st")
                    nc.sync.dma_start(st[:], slots[t])
                    ps = segsum(t, gts, st, hid_f)

                    h1 = wpool.tile([TILE, hid_f], f32, tag="h1")
                    nc.vector.tensor_scalar(
                        out=h1[:], in0=ps[:], scalar1=sc1_t[:, t:t + 1],
                        scalar2=None, op0=mybir.AluOpType.mult,
                    )
                    if has_b1:
                        bb = wpool.tile([TILE, hid_f], f32, tag="bb")
                        nc.vector.tensor_scalar(
                            out=bb[:], in0=b1_t[:], scalar1=soutc_t[:, t:t + 1],
                            scalar2=None, op0=mybir.AluOpType.mult,
                        )
                        nc.vector.tensor_tensor(
                            out=h1[:], in0=h1[:], in1=bb[:],
                            op=mybir.AluOpType.add,
                        )
                    nc.vector.tensor_scalar_max(h1[:], h1[:], 0.0)

                    pt = psB.tile([TILE, TILE], f32, tag="mm")
                    nc.tensor.transpose(pt[:], h1[:], ident_t[:])
                    h1T = wpool.tile([TILE, TILE], f32, tag="h1T")
                    nc.scalar.copy(h1T[:], pt[:])
                    p2 = psB.tile([TILE, out_f], f32, tag="mm")
                    nc.tensor.matmul(p2[:], lhsT=h1T[:], rhs=w2_t[:],
                                     start=True, stop=True)
                    sh2 = wpool.tile([TILE, TILE], bf16, tag="sh2")
                    nc.vector.memset(sh2[:, out_f:TILE], 0.0)
                    nc.vector.tensor_copy(sh2[:, 0:out_f], p2[:])
                    nc.sync.dma_start(shard2[t * TILE:(t + 1) * TILE, :], sh2[:])

                nc.gpsimd.collective_compute(
                    "AllGather", mybir.AluOpType.bypass, replica_groups=groups,
                    ins=[shard2[:]], outs=[table2[:]],
                )

                # ---- L2 aggregate + outputs ----
                for t in range(tpc):
                    gts = gather_tile(t, table2)
                    st = io.tile([TILE, 2 * K], f32, tag="st")
                    nc.sync.dma_start(st[:], slots[t])
                    ps = segsum(t, gts, st, out_f)

                    hid = wpool.tile([TILE, out_f], f32, tag="hid")
                    nc.vector.tensor_scalar(
                        out=hid[:], in0=ps[:], scalar1=sin_t[:, t:t + 1],
                        scalar2=None, op0=mybir.AluOpType.mult,
                    )
                    if has_b2:
                        nc.vector.tensor_tensor(
                            out=hid[:], in0=hid[:], in1=b2_t[:],
                            op=mybir.AluOpType.add,
                        )
                    nc.sync.dma_start(hid_out[t * TILE:(t + 1) * TILE, :], hid[:])

                    ptl = psB.tile([out_f, TILE], f32, tag="mmT")
                    nc.tensor.transpose(ptl[:], hid[:], ident_t[:])
                    hidT = wpool.tile([out_f, TILE], f32, tag="hidT")
                    nc.scalar.copy(hidT[:], ptl[:])
                    pl = psB.tile([TILE, lgt_f], f32, tag="mmL")
                    nc.tensor.matmul(pl[:], lhsT=hidT[:], rhs=wf_t[:],
                                     start=True, stop=True)
                    lg = wpool.tile([TILE, lgt_f], f32, tag="lg")
                    if has_bf:
                        nc.vector.tensor_tensor(
                            out=lg[:], in0=pl[:], in1=bf_t[:],
                            op=mybir.AluOpType.add,
                        )
                    else:
                        nc.vector.tensor_copy(lg[:], pl[:])
                    nc.sync.dma_start(log_out[t * TILE:(t + 1) * TILE, :], lg[:])

    nc.compile()
    return nc


# --------------------------------------------------------------------------
# entry point
# --------------------------------------------------------------------------

_PROG_CACHE = {}


def kernel(x, W1, b1, W2, b2, Wf, bf, src, dst):
    x = np.asarray(x)
    src = np.asarray(src)
    dst = np.asarray(dst)
    per_core, meta = host_prep(x, W1, b1, W2, b2, Wf, bf, src, dst)

    key = (meta["in_f"], meta["hid_f"], meta["out_f"], meta["spc"], meta["K"],
           meta["has_b1"], meta["has_b2"], meta["has_bf"])
    nc = _PROG_CACHE.get(key)
    if nc is None:
        nc = build_program(meta)
        _PROG_CACHE[key] = nc

    res = run_bass_kernel_spmd(nc, per_core, list(range(N_CORES)))

    hid_sl = np.concatenate([res.results[c]["hid_out"] for c in range(N_CORES)])
    log_sl = np.concatenate([res.results[c]["log_out"] for c in range(N_CORES)])
    slot_of = meta["slot_of"]
    hidden = hid_sl[slot_of].astype(np.float32)
    logits = log_sl[slot_of].astype(np.float32)
    return (logits, hidden)


# revision 5
# speedup vs baseline: 74.6710x; 74.6710x over previous
"""Two-layer GCN (contrastive head) on 8 Trainium2 NeuronCores.

Strategy (graph/data parallel per the node-partition sharding):
  * Host: compute degree normalizations, build a load-balancing node
    permutation (slots), partition edges by owner of dst, group per
    (dst-tile, src-half) into fixed-capacity chunk segments.
  * Device (SPMD on 8 cores):
      L1 transform:  h = (x * rsqrt(deg_out)) @ W1      (own nodes, PE fp32)
      AllGather bf16 h-table  -> every core holds the full table
      L1 aggregate:  dma_gather(h[src]) + one-hot segment-sum on PE
      h1 = relu(agg * rsqrt(deg_in) + b1); fold * rsqrt(deg_out)
      L2 transform:  t2 = h1s @ W2 ; AllGather bf16 t2-table
      L2 aggregate:  gather + segment-sum;  hidden = agg2 * rsqrt(deg_in) + b2
      logits = hidden @ Wf + bf
  * Host: concat per-core shards, un-permute, trim padding.
"""

import numpy as np
import ml_dtypes

import concourse.bacc as bacc
import concourse.tile as tile
from concourse import mybir
from concourse.bass_utils import run_bass_kernel_spmd

N_CORES = 8
TILE = 128
PAD_SLOT_VAL = 200.0  # one-hot compare value that never matches iota 0..127
GATHER_MAX_CHUNKS = 8   # 1024-idx gathers; fits the enlarged SWDGE ring
N_SWDGE_QUEUES = 4      # spread gathers across all 4 Q7 queue pairs
DMA_SCRATCH = 49152     # 3x default SWDGE descriptor-ring carveout


# --------------------------------------------------------------------------
# host-side preprocessing
# --------------------------------------------------------------------------

def _balance_slots(deg_in, n_bins):
    """Assign each node to a (core,tile) bin of capacity TILE, balancing
    total in-degree per bin (greedy LPT).  Returns slot_of[node]."""
    import heapq

    n = deg_in.shape[0]
    order = np.argsort(-deg_in, kind="stable")
    heap = [(0, b) for b in range(n_bins)]
    heapq.heapify(heap)
    counts = np.zeros(n_bins, np.int64)
    slot_of = np.empty(n, np.int64)
    for v in order:
        load, b = heapq.heappop(heap)
        slot_of[v] = b * TILE + counts[b]
        counts[b] += 1
        if counts[b] < TILE:
            heapq.heappush(heap, (load + int(deg_in[v]), b))
    return slot_of


def _wrap_idx(flat):
    """[n_seg, cap] int -> SWDGE wrapped layout [n_seg, 128, cap//16] int16
    (index i at partition i%16, column i//16; replicated x8 down partitions)."""
    n_seg, cap = flat.shape
    w = flat.reshape(n_seg, cap // 16, 16).transpose(0, 2, 1).astype(np.int16)
    return np.tile(w, (1, 8, 1))


def host_prep(x, W1, b1, W2, b2, Wf, bf, src, dst):
    n, in_f = x.shape
    hid_f = W1.shape[1]
    out_f = W2.shape[1]
    spc = -(-n // (N_CORES * TILE)) * TILE          # slots per core
    S = N_CORES * spc
    half = S // 2
    tpc = spc // TILE                                # tiles per core

    deg_out = np.bincount(src, minlength=n)
    deg_in = np.bincount(dst, minlength=n)
    s_out = 1.0 / np.sqrt(np.maximum(deg_out, 1)).astype(np.float32)
    s_in = 1.0 / np.sqrt(np.maximum(deg_in, 1)).astype(np.float32)

    slot_of = _balance_slots(deg_in, S // TILE)

    ds = slot_of[dst]
    ss = slot_of[src]
    half_e = (ss >= half).astype(np.int64)
    gidx = ss - half_e * half
    seg = (ds // TILE) * 2 + half_e                  # global segment id
    n_seg = (S // TILE) * 2

    cnt = np.bincount(seg, minlength=n_seg)
    K = max(1, -(-int(cnt.max()) // TILE))           # chunks per segment
    cap = K * TILE

    starts = np.zeros(n_seg + 1, np.int64)
    np.cumsum(cnt, out=starts[1:])
    eorder = np.argsort(seg, kind="stable")
    pos = np.arange(len(src)) - starts[seg[eorder]]

    idx_pad = np.zeros((n_seg, cap), np.int64)
    slot_pad = np.full((n_seg, cap), PAD_SLOT_VAL, np.float32)
    idx_pad[seg[eorder], pos] = gidx[eorder]
    slot_pad[seg[eorder], pos] = (ds % TILE)[eorder]

    # per-core device inputs
    xs = (x * s_out[:, None]).astype(np.float32)
    xp = np.zeros((S, in_f), np.float32)
    xp[slot_of] = xs
    s_in_sl = np.ones(S, np.float32)
    s_in_sl[slot_of] = s_in
    s_out_sl = np.ones(S, np.float32)
    s_out_sl[slot_of] = s_out
    sc1_sl = s_in_sl * s_out_sl

    iota = np.tile(np.arange(TILE, dtype=np.float32), (TILE, 1)).astype(
        ml_dtypes.bfloat16
    )
    ident = np.eye(TILE, dtype=np.float32)

    per_core = []
    for c in range(N_CORES):
        seg_lo, seg_hi = c * tpc * 2, (c + 1) * tpc * 2
        m = {
            "xT": np.ascontiguousarray(xp[c * spc:(c + 1) * spc].T),
            "w1": np.asarray(W1, np.float32),
            "w2": np.asarray(W2, np.float32),
            "wf": np.asarray(Wf, np.float32),
            "gidx": np.ascontiguousarray(_wrap_idx(idx_pad[seg_lo:seg_hi])),
            "slots": np.ascontiguousarray(
                slot_pad[seg_lo:seg_hi]
                .reshape(tpc, 2, K, TILE)
                .transpose(0, 3, 1, 2)
                .reshape(tpc, TILE, 2 * K)
                .astype(np.float32)
            ),
            "sin": np.ascontiguousarray(
                s_in_sl[c * spc:(c + 1) * spc].reshape(tpc, TILE).T
            ),
            "sc1": np.ascontiguousarray(
                sc1_sl[c * spc:(c + 1) * spc].reshape(tpc, TILE).T
            ),
            "iota": iota,
            "ident": ident,
        }
        if np.any(b1):
            m["b1t"] = np.tile(np.asarray(b1, np.float32), (TILE, 1))
            m["soutc"] = np.ascontiguousarray(
                s_out_sl[c * spc:(c + 1) * spc].reshape(tpc, TILE).T
            )
        if np.any(b2):
            m["b2t"] = np.tile(np.asarray(b2, np.float32), (TILE, 1))
        if np.any(bf):
            m["bft"] = np.tile(np.asarray(bf, np.float32), (TILE, 1))
        per_core.append(m)

    meta = dict(
        n=n, in_f=in_f, hid_f=hid_f, out_f=out_f, spc=spc, S=S, half=half,
        tpc=tpc, K=K, slot_of=slot_of,
        has_b1=bool(np.any(b1)), has_b2=bool(np.any(b2)), has_bf=bool(np.any(bf)),
    )
    return per_core, meta


# --------------------------------------------------------------------------
# device program
# --------------------------------------------------------------------------

def build_program(meta, reps=1):
    in_f, hid_f, out_f = meta["in_f"], meta["hid_f"], meta["out_f"]
    spc, S, half, tpc, K = (
        meta["spc"], meta["S"], meta["half"], meta["tpc"], meta["K"],
    )
    has_b1, has_b2, has_bf = meta["has_b1"], meta["has_b2"], meta["has_bf"]
    lgt_f = 2
    kin = in_f // TILE                      # k-tiles of the L1 transform
    assert in_f % TILE == 0 and hid_f == TILE and out_f <= TILE

    f32, bf16, i16 = mybir.dt.float32, mybir.dt.bfloat16, mybir.dt.int16
    nc = bacc.Bacc("TRN2", target_bir_lowering=False, debug=False,
                   num_devices=N_CORES, num_swdge_queues=N_SWDGE_QUEUES,
                   dynamic_dma_scratch_size=DMA_SCRATCH)

    def din(name, shape, dt):
        return nc.dram_tensor(name, shape, dt, kind="ExternalInput").ap()

    xT = din("xT", [in_f, spc], f32)
    w1 = din("w1", [in_f, hid_f], f32)
    w2 = din("w2", [hid_f, out_f], f32)
    wf = din("wf", [out_f, lgt_f], f32)
    gidx = din("gidx", [2 * tpc, TILE, K * 8], i16)
    slots = din("slots", [tpc, TILE, 2 * K], f32)
    sin_d = din("sin", [TILE, tpc], f32)
    sc1_d = din("sc1", [TILE, tpc], f32)
    iota_d = din("iota", [TILE, TILE], bf16)
    ident_d = din("ident", [TILE, TILE], f32)
    b1t_d = din("b1t", [TILE, hid_f], f32) if has_b1 else None
    soutc_d = din("soutc", [TILE, tpc], f32) if has_b1 else None
    b2t_d = din("b2t", [TILE, out_f], f32) if has_b2 else None
    bft_d = din("bft", [TILE, lgt_f], f32) if has_bf else None

    hid_out = nc.dram_tensor("hid_out", [spc, out_f], f32,
                             kind="ExternalOutput").ap()
    log_out = nc.dram_tensor("log_out", [spc, lgt_f], f32,
                             kind="ExternalOutput").ap()

    shard1 = nc.dram_tensor("shard1", [spc, hid_f], bf16).ap()
    table1 = nc.dram_tensor("table1", [S, hid_f], bf16, addr_space="Shared").ap()
    shard2 = nc.dram_tensor("shard2", [spc, TILE], bf16).ap()
    table2 = nc.dram_tensor("table2", [S, TILE], bf16, addr_space="Shared").ap()
    groups = [list(range(N_CORES))]

    # gather spans per segment (split if K exceeds the SWDGE ring capacity)
    spans = []
    j0 = 0
    while j0 < K:
        j1 = min(j0 + GATHER_MAX_CHUNKS, K)
        spans.append((j0, j1))
        j0 = j1

    with tile.TileContext(nc) as tc:
        with (
            tc.tile_pool(name="const", bufs=1) as cpool,
            tc.tile_pool(name="io", bufs=3) as io,
            tc.tile_pool(name="gth", bufs=3) as gpool,
            tc.tile_pool(name="onehot", bufs=4) as spool,
            tc.tile_pool(name="work", bufs=3) as wpool,
            tc.tile_pool(name="psA", bufs=2, space="PSUM") as psA,
            tc.tile_pool(name="psB", bufs=2, space="PSUM") as psB,
        ):
            iota_t = cpool.tile([TILE, TILE], bf16, tag="iota")
            nc.sync.dma_start(iota_t[:], iota_d[:])
            ident_t = cpool.tile([TILE, TILE], f32, tag="ident")
            nc.sync.dma_start(ident_t[:], ident_d[:])
            w1_t = []
            for k in range(kin):
                t = cpool.tile([TILE, hid_f], f32, tag=f"w1_{k}")
                nc.sync.dma_start(t[:], w1[k * TILE:(k + 1) * TILE, :])
                w1_t.append(t)
            w2_t = cpool.tile([hid_f, out_f], f32, tag="w2")
            nc.sync.dma_start(w2_t[:], w2[:])
            wf_t = cpool.tile([out_f, lgt_f], f32, tag="wf")
            nc.sync.dma_start(wf_t[:], wf[:])
            sin_t = cpool.tile([TILE, tpc], f32, tag="sin")
            nc.sync.dma_start(sin_t[:], sin_d[:])
            sc1_t = cpool.tile([TILE, tpc], f32, tag="sc1")
            nc.sync.dma_start(sc1_t[:], sc1_d[:])
            if has_b1:
                b1_t = cpool.tile([TILE, hid_f], f32, tag="b1")
                nc.sync.dma_start(b1_t[:], b1t_d[:])
                soutc_t = cpool.tile([TILE, tpc], f32, tag="soutc")
                nc.sync.dma_start(soutc_t[:], soutc_d[:])
            if has_b2:
                b2_t = cpool.tile([TILE, out_f], f32, tag="b2")
                nc.sync.dma_start(b2_t[:], b2t_d[:])
            if has_bf:
                bf_t = cpool.tile([TILE, lgt_f], f32, tag="bf")
                nc.sync.dma_start(bf_t[:], bft_d[:])

            qctr = [0]

            def gather_tile(t, table):
                """Issue the 2 half-gathers for dst-tile t. Returns tiles."""
                gts = []
                for h in (0, 1):
                    it = io.tile([TILE, K * 8], i16, tag="it")
                    nc.sync.dma_start(it[:], gidx[2 * t + h])
                    gt = gpool.tile([TILE, K, TILE], bf16, tag="gt")
                    for (j0, j1) in spans:
                        nidx = (j1 - j0) * TILE
                        nc.gpsimd.dma_gather(
                            gt[:, j0:j1, :],
                            table[h * half:(h + 1) * half, :],
                            it[:, j0 * 8:j1 * 8],
                            nidx, nidx, TILE,
                            queue_num=qctr[0] % N_SWDGE_QUEUES,
                        )
                        qctr[0] += 1
                    gts.append(gt)
                return gts

            def segsum(t, gts, st, fdim):
                """PSUM accumulate one-hot.T @ gathered over 2K chunks."""
                ps = psA.tile([TILE, fdim], f32, tag="agg")
                for j in range(2 * K):
                    s_t = spool.tile([TILE, TILE], bf16, tag="oh")
                    nc.vector.tensor_scalar(
                        out=s_t[:], in0=iota_t[:], scalar1=st[:, j:j + 1],
                        scalar2=None, op0=mybir.AluOpType.is_equal,
                    )
                    nc.tensor.matmul(
                        ps[:], lhsT=s_t[:], rhs=gts[j // K][:, j % K, 0:fdim],
                        start=(j == 0), stop=(j == 2 * K - 1),
                    )
                return ps

            for _ in range(reps):
                # ---- L1 transform: own nodes ----
                for t in range(tpc):
                    ps = psB.tile([TILE, hid_f], f32, tag="mm")
                    for k in range(kin):
                        xt = io.tile([TILE, TILE], f32, tag="xt")
                        nc.sync.dma_start(
                            xt[:],
                            xT[k * TILE:(k + 1) * TILE, t * TILE:(t + 1) * TILE],
                        )
                        nc.tensor.matmul(ps[:], lhsT=xt[:], rhs=w1_t[k][:],
                                         start=(k == 0), stop=(k == kin - 1))
                    sh = wpool.tile([TILE, hid_f], bf16, tag="sh1")
                    nc.vector.tensor_copy(sh[:], ps[:])
                    nc.sync.dma_start(shard1[t * TILE:(t + 1) * TILE, :], sh[:])

                nc.gpsimd.collective_compute(
                    "AllGather", mybir.AluOpType.bypass, replica_groups=groups,
                    ins=[shard1[:]], outs=[table1[:]],
                )

                # ---- L1 aggregate + h1 + L2 transform ----
                for t in range(tpc):
                    gts = gather_tile(t, table1)
                    st = io.tile([TILE, 2 * K], f32, tag="st")
                    nc.sync.dma_start(st[:], slots[t])
                    ps = segsum(t, gts, st, hid_f)

                    h1 = wpool.tile([TILE, hid_f], f32, tag="h1")
                    nc.vector.tensor_scalar(
                        out=h1[:], in0=ps[:], scalar1=sc1_t[:, t:t + 1],
                        scalar2=None, op0=mybir.AluOpType.mult,
                    )
                    if has_b1:
                        bb = wpool.tile([TILE, hid_f], f32, tag="bb")
                        nc.vector.tensor_scalar(
                            out=bb[:], in0=b1_t[:], scalar1=soutc_t[:, t:t + 1],
                            scalar2=None, op0=mybir.AluOpType.mult,
                        )
                        nc.vector.tensor_tensor(
                            out=h1[:], in0=h1[:], in1=bb[:],
                            op=mybir.AluOpType.add,
                        )
                    nc.vector.tensor_scalar_max(h1[:], h1[:], 0.0)

                    pt = psB.tile([TILE, TILE], f32, tag="mm")
                    nc.tensor.transpose(pt[:], h1[:], ident_t[:])
                    h1T = wpool.tile([TILE, TILE], f32, tag="h1T")
                    nc.scalar.copy(h1T[:], pt[:])
                    p2 = psB.tile([TILE, out_f], f32, tag="mm")
                    nc.tensor.matmul(p2[:], lhsT=h1T[:], rhs=w2_t[:],
                                     start=True, stop=True)
                    sh2 = wpool.tile([TILE, TILE], bf16, tag="sh2")
                    nc.vector.memset(sh2[:, out_f:TILE], 0.0)
                    nc.vector.tensor_copy(sh2[:, 0:out_f], p2[:])
                    nc.sync.dma_start(shard2[t * TILE:(t + 1) * TILE, :], sh2[:])

                nc.gpsimd.collective_compute(
                    "AllGather", mybir.AluOpType.bypass, replica_groups=groups,
                    ins=[shard2[:]], outs=[table2[:]],
                )

                # ---- L2 aggregate + outputs ----
                for t in range(tpc):
                    gts = gather_tile(t, table2)
                    st = io.tile([TILE, 2 * K], f32, tag="st")
                    nc.sync.dma_start(st[:], slots[t])
                    ps = segsum(t, gts, st, out_f)

                    hid = wpool.tile([TILE, out_f], f32, tag="hid")
                    nc.vector.tensor_scalar(
                        out=hid[:], in0=ps[:], scalar1=sin_t[:, t:t + 1],
                        scalar2=None, op0=mybir.AluOpType.mult,
                    )
                    if has_b2:
                        nc.vector.tensor_tensor(
                            out=hid[:], in0=hid[:], in1=b2_t[:],
                            op=mybir.AluOpType.add,
                        )
                    nc.sync.dma_start(hid_out[t * TILE:(t + 1) * TILE, :], hid[:])

                    ptl = psB.tile([out_f, TILE], f32, tag="mmT")
                    nc.tensor.transpose(ptl[:], hid[:], ident_t[:])
                    hidT = wpool.tile([out_f, TILE], f32, tag="hidT")
                    nc.scalar.copy(hidT[:], ptl[:])
                    pl = psB.tile([TILE, lgt_f], f32, tag="mmL")
                    nc.tensor.matmul(pl[:], lhsT=hidT[:], rhs=wf_t[:],
                                     start=True, stop=True)
                    lg = wpool.tile([TILE, lgt_f], f32, tag="lg")
                    if has_bf:
                        nc.vector.tensor_tensor(
                            out=lg[:], in0=pl[:], in1=bf_t[:],
                            op=mybir.AluOpType.add,
                        )
                    else:
                        nc.vector.tensor_copy(lg[:], pl[:])
                    nc.sync.dma_start(log_out[t * TILE:(t + 1) * TILE, :], lg[:])

    nc.compile()
    return nc


# --------------------------------------------------------------------------
# entry point
# --------------------------------------------------------------------------

_PROG_CACHE = {}


def kernel(x, W1, b1, W2, b2, Wf, bf, src, dst):
    x = np.asarray(x)
    src = np.asarray(src)
    dst = np.asarray(dst)
    per_core, meta = host_prep(x, W1, b1, W2, b2, Wf, bf, src, dst)

    key = (meta["in_f"], meta["hid_f"], meta["out_f"], meta["spc"], meta["K"],
           meta["has_b1"], meta["has_b2"], meta["has_bf"])
    nc = _PROG_CACHE.get(key)
    if nc is None:
        nc = build_program(meta)
        _PROG_CACHE[key] = nc

    res = run_bass_kernel_spmd(nc, per_core, list(range(N_CORES)))

    hid_sl = np.concatenate([res.results[c]["hid_out"] for c in range(N_CORES)])
    log_sl = np.concatenate([res.results[c]["log_out"] for c in range(N_CORES)])
    slot_of = meta["slot_of"]
    hidden = hid_sl[slot_of].astype(np.float32)
    logits = log_sl[slot_of].astype(np.float32)
    return (logits, hidden)
